# revision 54
# baseline (speedup 1.0000x reference)
"""Multi-head causal attention with RoPE on 8 Trainium2 NeuronCores.

Sharding: core c -> (batch b = c//4, head-group g = c%4, heads 4g..4g+4).
wq/wk/wv column-sharded by head, wo row-sharded; attention fully local.
Host sums the per-core partial output projections per batch.

Numerics: matmul operands bf16, accumulation fp32 (PSUM); RoPE tables and
softmax statistics fp32 -> bf16 tables; y emitted as two bf16 partial sums
(one per 128-row j-tile), summed in fp32 on host.

Schedule: trig tables are computed first on DVE/ACT with their small DMAs on
the ACT hardware-DGE queue so they never sit behind the 6.5MB of input loads
on the SP queue.  x/w are DMA'd in k-chunks with the k/q projections of
head-pair 0 chasing the DMA k-chunk-major; RoPE runs per 512-column chunk so
attention starts as soon as chunk 0 of kT/qT is roped.  The v projection and
the pair-1 k/q projections execute in pair-0 attention bubbles; the output
projection of j-tile 0 and most of j-tile 1 execute in pair-1 bubbles.
Attention is column-chunk-major (per 512-wide output chunk, accumulate over
all key tiles) so the softmax denominator finishes with the chunk and
normalization is fused into the PSUM unload.  Both heads of a pair share one
[128,1024] exp instruction and one causal-mask instruction.
"""
import sys
sys.path.insert(0, "/opt/trn_rl_repo")
import numpy as np
from ml_dtypes import bfloat16

import concourse.bass as bass
import concourse.tile as tile
from concourse import bacc, mybir
from concourse.bass_utils import run_bass_kernel_spmd

F = mybir.ActivationFunctionType
A = mybir.AluOpType
FP32 = mybir.dt.float32
BF16 = mybir.dt.bfloat16
I32 = mybir.dt.int32

B, D, H = 2, 1024, 16
NCORES = 8
GROUPS = 4            # head groups (cores per batch)
HL = H // GROUPS      # heads per core = 4
DK = D // H           # 64
JL = HL * DK          # local projection width = 256
ROPE_THETA = 10000.0
TWO_PI = 2 * np.pi


def build_mha(S: int, max_phase: int = 9, reps: int = 1):
    """One SPMD program: per-core shard of the full MHA layer."""
    assert S % 512 == 0
    NT = S // 128          # 128-tiles along sequence
    NC = S // 512          # 512-chunks along sequence
    KT = D // 128          # 8 contraction tiles for projections
    SCALE = 1.0 / np.sqrt(DK)

    nc = bacc.Bacc(None, target_bir_lowering=False, debug=False)

    xt_in = nc.declare_dram_parameter("xt", [D, S], BF16, isOutput=False)
    wq_in = nc.declare_dram_parameter("wqt", [D, JL], BF16, isOutput=False)
    wk_in = nc.declare_dram_parameter("wkt", [D, JL], BF16, isOutput=False)
    wv_in = nc.declare_dram_parameter("wvt", [D, JL], BF16, isOutput=False)
    wo_in = nc.declare_dram_parameter("wot", [JL, D], BF16, isOutput=False)
    pos_in = nc.declare_dram_parameter("pos", [1, S], I32, isOutput=False)
    ivt_in = nc.declare_dram_parameter("ivturns", [DK, 1], FP32, isOutput=False)
    alt_in = nc.declare_dram_parameter("altsign", [DK, 1], FP32, isOutput=False)
    perm_in = nc.declare_dram_parameter("perm", [128, 128], BF16, isOutput=False)
    y_out = nc.declare_dram_parameter("y", [2, S, D], BF16, isOutput=True)

    with tile.TileContext(nc) as tc:
        persist = tc.alloc_tile_pool(name="persist", bufs=1)
        kTb = [persist.tile([128, S], BF16, tag=f"kTb{i}", name=f"kTb{i}") for i in range(2)]
        qTb = [persist.tile([128, S], BF16, tag=f"qTb{i}", name=f"qTb{i}") for i in range(2)]
        v_sb = persist.tile([128, NT, HL, DK + 1], BF16, tag="v")
        attnT = persist.tile([128, 2, S], BF16, tag="attnT")
        woTb = persist.tile([128, 2, D], BF16, tag="woTb")
        cos128 = persist.tile([128, S], BF16, tag="cos128")
        sinalt128 = persist.tile([128, S], BF16, tag="sinalt128")
        ones64 = persist.tile([1, DK], BF16, tag="ones64")

        nc.vector.memset(ones64, 1.0)
        perm = persist.tile([128, 128], BF16, tag="perm")
        expwarm = persist.tile([1, 16], FP32, tag="expwarm")
        nc.vector.memset(expwarm, 0.0)
        nc.sync.dma_start(out=perm, in_=perm_in[:, :])

        for _rep in range(reps):
          with tc.tile_pool(name="wx", bufs=2) as wx, \
               tc.tile_pool(name="ps", bufs=3, space="PSUM") as ps, \
               tc.tile_pool(name="ov", bufs=2, space="PSUM") as ovp, \
               tc.tile_pool(name="esr", bufs=6) as esrp, \
               tc.tile_pool(name="rope", bufs=2) as ropep, \
               tc.tile_pool(name="rs", bufs=6) as rsp, \
               tc.tile_pool(name="yst", bufs=4) as ystp:
            # ---- trig tables first; their DMAs ride the ACT hardware queue ----
            ivt = wx.tile([DK, 1], FP32, tag="ivt")
            alt = wx.tile([DK, 1], FP32, tag="alt")
            # Two passes (cols 0:512 first) so rope of chunk 0 is unblocked in
            # a few us; fractional turns via floor-mod, sin/cos on ACT.
            with tc.tile_pool(name="trig", bufs=1) as trig:
                posb_i = trig.tile([DK, S], I32, tag="posbi")
                pap = pos_in[:, :]
                nc.scalar.dma_start(
                    out=posb_i,
                    in_=bass.AP(tensor=pap.tensor, offset=pap.offset,
                                ap=[[0, DK]] + list(pap.ap)[1:]))
                nc.scalar.dma_start(out=ivt, in_=ivt_in[:, :])
                nc.scalar.dma_start(out=alt, in_=alt_in[:, :])
                f = trig.tile([DK, S], FP32, tag="f")
                msk = trig.tile([DK, S], FP32, tag="msk")
                ki = trig.tile([DK, S], I32, tag="ki")
                for lo, hi in ((0, 512), (512, S)):
                    cs = slice(lo, hi)
                    fc, mc, kc = f[:, cs], msk[:, cs], ki[:, cs]
                    nc.vector.tensor_copy(out=fc, in_=posb_i[:, cs])
                    nc.vector.tensor_scalar(fc, fc, ivt, None, op0=A.mult)
                    nc.vector.tensor_copy(out=kc, in_=fc)   # whole turns
                    nc.vector.tensor_copy(out=mc, in_=kc)
                    nc.vector.tensor_tensor(fc, fc, mc, op=A.subtract)
                    nc.vector.tensor_scalar(mc, fc, 0.5, None, op0=A.is_gt)
                    nc.vector.scalar_tensor_tensor(fc, mc, -1.0, fc,
                                                   op0=A.mult, op1=A.add)
                    s64 = trig.tile([DK, 512], FP32, tag="s64", name=f"s64{lo}")
                    for c0 in range(lo, hi, 512):
                        c0s = slice(c0, c0 + 512)
                        nc.scalar.activation(out=s64, in_=f[:, c0s],
                                             func=F.Sin, scale=TWO_PI)
                        sa = trig.tile([DK, 512], BF16, tag="sa", name=f"sa{c0}")
                        nc.vector.tensor_scalar(sa, s64, alt, None, op0=A.mult)
                        nc.scalar.dma_start(out=sinalt128[0:DK, c0s], in_=sa)
                        nc.scalar.dma_start(out=sinalt128[DK:128, c0s], in_=sa)
                    # cos(2pi f) = sin(2pi (f + 1/4 wrapped))
                    nc.vector.tensor_scalar(fc, fc, 0.25, None, op0=A.add)
                    nc.vector.tensor_scalar(mc, fc, 0.5, None, op0=A.is_gt)
                    nc.vector.scalar_tensor_tensor(fc, mc, -1.0, fc,
                                                   op0=A.mult, op1=A.add)
                    for c0 in range(lo, hi, 512):
                        c0s = slice(c0, c0 + 512)
                        c64 = trig.tile([DK, 512], BF16, tag="c64", name=f"c64{c0}")
                        nc.scalar.activation(out=c64, in_=f[:, c0s],
                                             func=F.Sin, scale=TWO_PI)
                        nc.scalar.dma_start(out=cos128[0:DK, c0s], in_=c64)
                        nc.scalar.dma_start(out=cos128[DK:128, c0s], in_=c64)

            # ---- staged input DMAs: interleaved wk/wq/x k-chunks on SP queue ----
            xtb = wx.tile([128, KT, S], BF16, tag="xtb")
            wb = {}
            for name in ("q", "k", "v"):
                wb[name] = wx.tile([128, KT, JL], BF16, tag=f"wb{name}", name=f"wb{name}")
            for kk in range(KT // 2):
                r = slice(256 * kk, 256 * (kk + 1))
                nc.sync.dma_start(out=wb["k"][:, 2 * kk:2 * kk + 2, :],
                                  in_=wk_in[r, :].rearrange("(k p) j -> p k j", p=128))
                nc.sync.dma_start(out=wb["q"][:, 2 * kk:2 * kk + 2, :],
                                  in_=wq_in[r, :].rearrange("(k p) j -> p k j", p=128))
                nc.sync.dma_start(out=xtb[:, 2 * kk:2 * kk + 2, :],
                                  in_=xt_in[r, :].rearrange("(k p) s -> p k s", p=128))
            nc.sync.dma_start(out=wb["v"],
                              in_=wv_in[:, :].rearrange("(k p) j -> p k j", p=128))
            nc.sync.dma_start(out=woTb,
                              in_=wo_in[:, :].rearrange("(t p) e -> p t e", p=128))

            # ---- RoPE in three steps: per-chunk psum unload, one swap DMA
            # pair (partition 32-blocks exchanged pairwise), per-chunk muls ----
            def rope_copy(t16, psum, sc, on_act):
                if on_act:
                    nc.scalar.activation(out=t16[:, 512 * sc:512 * (sc + 1)],
                                         in_=psum, func=F.Copy)
                else:
                    nc.vector.tensor_copy(out=t16[:, 512 * sc:512 * (sc + 1)],
                                          in_=psum)

            def rope_muls(dst, t16, pph, sc):
                cols = slice(512 * sc, 512 * (sc + 1))
                tmp = ropep.tile([128, 512], BF16, tag="ropetmp")
                swb = ropep.tile([128, 512], BF16, tag="swb")
                nc.gpsimd.tensor_mul(tmp, t16[:, cols], cos128[:, cols])
                nc.vector.tensor_mul(swb, sinalt128[:, cols], pph)
                nc.vector.tensor_add(dst[:, cols], tmp, swb)

            # ---- k/q of j-tile 0, k-chunk-major so matmuls chase the x DMA ----
            # PSUM alloc order matches the pp-reuse order below so the rope
            # perm tiles recycle slots whose unload copies are already emitted.
            kps = [ps.tile([128, 1024], FP32, tag="ps", name=f"k0ps{i}") for i in range(2)]
            qps01 = ps.tile([128, 1024], FP32, tag="ps", name="q0ps01")
            qps23 = [ovp.tile([128, 512], FP32, tag="ov", name=f"q0ps{i}") for i in range(2)]
            kq0 = {"k": [kps[sc // 2][:, 512 * (sc % 2):512 * (sc % 2 + 1)] for sc in range(NC)],
                   "q": [qps01[:, 0:512], qps01[:, 512:1024], qps23[0][:, :], qps23[1][:, :]]}
            for kk in range(KT // 2):
                for name in ("k", "q"):
                    for sc in range(NC):
                        for dk_ in range(2):
                            k = 2 * kk + dk_
                            nc.tensor.matmul(
                                out=kq0[name][sc],
                                lhsT=wb[name][:, k, 0:128],
                                rhs=xtb[:, k, 512 * sc:512 * (sc + 1)],
                                start=(k == 0), stop=(k == KT - 1))
            nc.scalar.activation(out=expwarm, in_=expwarm, func=F.Exp)  # table preload
            for name, dst in (("k", kTb[0]), ("q", qTb[0])):
                t16 = ropep.tile([128, S], BF16, tag="t16", name=f"t16{name}0")
                for sc in range(NC):
                    rope_copy(t16, kq0[name][sc], sc, on_act=(name == "k"))
                for scp in range(NC // 2):
                    pp = ps.tile([128, 1024], FP32, tag="ps", name=f"rp{name}0{scp}")
                    for half in range(2):
                        sc = 2 * scp + half
                        nc.tensor.matmul(out=pp[:, 512 * half:512 * (half + 1)],
                                         lhsT=perm,
                                         rhs=t16[:, 512 * sc:512 * (sc + 1)],
                                         start=True, stop=True)
                    for half in range(2):
                        sc = 2 * scp + half
                        rope_muls(dst, t16, pp[:, 512 * half:512 * (half + 1)], sc)

            # ---- deferred work generators (run inside attention bubbles) ----
            def v_tile(st):
                vps = ps.tile([128, 1024], FP32, tag="ps", name=f"vps{st}")
                for k in range(KT):
                    nc.tensor.matmul(out=vps[:, 0:JL],
                                     lhsT=xtb[:, k, 128 * st:128 * (st + 1)],
                                     rhs=wb["v"][:, k, :],
                                     start=(k == 0), stop=(k == KT - 1))
                nc.vector.tensor_copy(
                    out=v_sb[:, st, :, 0:DK],
                    in_=vps[:, 0:JL].rearrange("p (h d) -> p h d", h=HL))

            t16j1 = {}

            def kq1_half(name, sp_):
                # project j-tile 1 chunks 2*sp_, 2*sp_+1; rope after second half
                if name not in t16j1:
                    t16j1[name] = ropep.tile([128, S], BF16, tag="t16",
                                             name=f"t16{name}1")
                t16 = t16j1[name]
                t = ps.tile([128, 1024], FP32, tag="ps", name=f"kq1{name}{sp_}")
                dst = kTb[1] if name == "k" else qTb[1]
                for half in range(2):
                    sc = 2 * sp_ + half
                    for k in range(KT):
                        nc.tensor.matmul(
                            out=t[:, 512 * half:512 * (half + 1)],
                            lhsT=wb[name][:, k, 128:256],
                            rhs=xtb[:, k, 512 * sc:512 * (sc + 1)],
                            start=(k == 0), stop=(k == KT - 1))
                for half in range(2):
                    rope_copy(t16, t[:, 512 * half:512 * (half + 1)],
                              2 * sp_ + half, on_act=False)
                if sp_ == 1:
                    for scp in range(NC // 2):
                        pp = ps.tile([128, 1024], FP32, tag="ps",
                                     name=f"rp1{name}{scp}")
                        for half in range(2):
                            sc = 2 * scp + half
                            nc.tensor.matmul(out=pp[:, 512 * half:512 * (half + 1)],
                                             lhsT=perm,
                                             rhs=t16[:, 512 * sc:512 * (sc + 1)],
                                             start=True, stop=True)
                        for half in range(2):
                            sc = 2 * scp + half
                            rope_muls(dst, t16, pp[:, 512 * half:512 * (half + 1)], sc)

            def out_proj(jt, st, tail=False):
                po = ps.tile([128, 1024], FP32, tag="ps", name=f"po{jt}{st}")
                for ec in range(2):
                    nc.tensor.matmul(
                        out=po[:, 512 * ec:512 * (ec + 1)],
                        lhsT=attnT[:, jt, 128 * st:128 * (st + 1)],
                        rhs=woTb[:, jt, 512 * ec:512 * (ec + 1)],
                        start=True, stop=True)
                yst = ystp.tile([128, 1024], BF16, tag="yst")
                if tail or st % 2 == 1:
                    nc.scalar.activation(out=yst[:, 0:512], in_=po[:, 0:512], func=F.Copy)
                    nc.vector.tensor_copy(out=yst[:, 512:1024], in_=po[:, 512:1024])
                else:
                    nc.vector.tensor_copy(out=yst, in_=po)
                nc.sync.dma_start(out=y_out[jt, 128 * st:128 * (st + 1), :], in_=yst)

            # ---- attention: one pair of heads (2jt, 2jt+1) ----
            def attention_pair(jt, fillers):
                kT, qT = kTb[jt], qTb[jt]
                fill_i = 0

                def fill():
                    nonlocal fill_i
                    if fill_i < len(fillers):
                        fillers[fill_i]()
                        fill_i += 1

                for c in range(NC):
                    nmi = 4 * c + 4
                    ov = [ovp.tile([DK + 1, 512], FP32, tag="ov", name=f"ov{jt}{c}{x}")
                          for x in range(2)]
                    pend = None

                    def emit_pv(mi, esr, o):
                        for x in range(2):
                            nc.tensor.matmul(
                                out=ov[x][:, o:512],
                                lhsT=v_sb[:, mi, 2 * jt + x, :],
                                rhs=esr[:, x, o:512],
                                start=(mi == 0), stop=(mi == nmi - 1))

                    for mi in range(nmi):
                        o = max(0, 128 * mi - 512 * c)   # diagonal offset within chunk
                        sp = ps.tile([128, 1024], FP32, tag="ps", name=f"sp{jt}{c}{mi}")
                        sp2 = sp.rearrange("p (x n) -> p x n", x=2)
                        for x in range(2):
                            pb = 64 * x
                            nc.tensor.matmul(
                                out=sp2[:, x, o:512],
                                lhsT=kT[pb:pb + DK, 128 * mi:128 * (mi + 1)],
                                rhs=qT[pb:pb + DK, 512 * c + o:512 * (c + 1)],
                                start=True, stop=True)
                        esr = esrp.tile([128, 2, 512], BF16, tag="esr")
                        nc.scalar.activation(out=esr[:, :, o:512], in_=sp2[:, :, o:512],
                                             func=F.Exp, scale=SCALE)
                        if mi >= 4 * c:                  # diagonal tile: mask n<m
                            for x in range(2):
                                nc.gpsimd.affine_select(
                                    out=esr[:, x, o:o + 128], in_=esr[:, x, o:o + 128],
                                    pattern=[[1, 128]], compare_op=A.is_ge,
                                    fill=0.0, base=0, channel_multiplier=-1)
                        fill()
                        if pend is not None:
                            emit_pv(*pend)
                        pend = (mi, esr, o)
                    emit_pv(*pend)
                    # retire chunk c: denom rows are final -> normalize into attnT
                    rs = [rsp.tile([1, 512], BF16, tag="rs", name=f"rs{jt}{c}{x}")
                          for x in range(2)]
                    with nc.allow_low_precision(reason="bf16 softmax denom recip"):
                        for x in range(2):
                            nc.vector.reciprocal(out=rs[x], in_=ov[x][DK:DK + 1, :])
                    bc = ps.tile([128, 1024], FP32, tag="ps", name=f"bc{jt}{c}")
                    for x in range(2):
                        nc.tensor.matmul(out=bc[DK * x:DK * (x + 1), 0:512],
                                         lhsT=ones64, rhs=rs[x],
                                         start=True, stop=True)
                    cols = slice(512 * c, 512 * (c + 1))
                    nc.vector.tensor_copy(out=attnT[0:DK, jt, cols], in_=ov[0][0:DK, :])
                    nc.vector.tensor_copy(out=attnT[DK:128, jt, cols], in_=ov[1][0:DK, :])
                    nc.vector.tensor_mul(attnT[0:DK, jt, cols],
                                         attnT[0:DK, jt, cols], bc[0:DK, 0:512])
                    nc.vector.tensor_mul(attnT[DK:128, jt, cols],
                                         attnT[DK:128, jt, cols], bc[DK:128, 0:512])

            # memset v denominator-indicator column once (before first PV)
            nc.vector.memset(v_sb[:, :, :, DK:DK + 1], 1.0)
            for st in range(4):
                v_tile(st)

            fillers0 = [lambda st=st: v_tile(st) for st in range(4, NT)]
            fillers0 += [lambda: kq1_half("k", 0), lambda: kq1_half("k", 1),
                         lambda: kq1_half("q", 0), lambda: kq1_half("q", 1)]
            attention_pair(0, fillers0)
            fillers1 = [lambda st=st: out_proj(0, st) for st in range(NT)]
            fillers1 += [lambda st=st: out_proj(1, st) for st in range(NT - 4)]
            attention_pair(1, fillers1)
            for st in range(NT - 4, NT):
                out_proj(1, st, tail=True)

        persist.release()

    nc.compile()
    return nc


_cache = {}

def _get_program(S):
    if S not in _cache:
        _cache[S] = build_mha(S)
    return _cache[S]


def make_in_maps(x, token_positions, wq, wk, wv, wo):
    S = x.shape[1]
    invfreq = ROPE_THETA ** (-np.arange(0, DK, 2, dtype=np.float32) / DK)
    ivturns = (np.concatenate([invfreq, invfreq]) / TWO_PI).astype(np.float32).reshape(DK, 1)
    altsign = np.concatenate([-np.ones(DK // 2), np.ones(DK // 2)]).astype(np.float32).reshape(DK, 1)
    # perm: within each 64-wide head block, evens first then odds
    blockperm = np.concatenate([np.arange(0, DK, 2), np.arange(1, DK, 2)])
    jperm = np.concatenate([64 * hh + blockperm for hh in range(HL)])
    permmat = np.zeros((128, 128), dtype=bfloat16)
    for i in range(128):
        permmat[i ^ 32, i] = 1.0

    in_maps = []
    for c in range(NCORES):
        b, g = c // GROUPS, c % GROUPS
        js = slice(JL * g, JL * (g + 1))
        in_maps.append({
            "xt": np.ascontiguousarray(x[b].T).astype(bfloat16),
            "wqt": np.ascontiguousarray(wq[js, :][jperm, :].T).astype(bfloat16),
            "wkt": np.ascontiguousarray(wk[js, :][jperm, :].T).astype(bfloat16),
            "wvt": np.ascontiguousarray(wv[js, :].T).astype(bfloat16),
            "wot": np.ascontiguousarray(wo[:, js].T).astype(bfloat16),
            "pos": np.asarray(token_positions[b], dtype=np.int32).reshape(1, S),
            "ivturns": ivturns,
            "altsign": altsign,
            "perm": permmat,
        })
    return in_maps


def kernel(x, token_positions, wq, wk, wv, wo):
    x = np.asarray(x, dtype=np.float32)
    token_positions = np.asarray(token_positions)
    wq = np.asarray(wq, dtype=np.float32)
    wk = np.asarray(wk, dtype=np.float32)
    wv = np.asarray(wv, dtype=np.float32)
    wo = np.asarray(wo, dtype=np.float32)
    S = x.shape[1]

    nc = _get_program(S)
    in_maps = make_in_maps(x, token_positions, wq, wk, wv, wo)
    res = run_bass_kernel_spmd(nc, in_maps, core_ids=list(range(NCORES)))
    out = np.zeros((B, S, D), dtype=np.float32)
    for c in range(NCORES):
        y = res.results[c]["y"].astype(np.float32)
        out[c // GROUPS] += y[0] + y[1]
    return out
